# revision 1
# baseline (speedup 1.0000x reference)
import numpy as np

import concourse.bacc as bacc
import concourse.tile as tile
import concourse.mybir as mybir
from concourse.bass_utils import run_bass_kernel_spmd

B, D, G, GS = 262144, 512, 4, 4
NO = G + G * GS
NCORES = 8
BC = B // NCORES
P = 128
S = BC // P
NBLK = S // 4
NHALF = S // 2
CH = D // P

F32 = mybir.dt.float32
I32 = mybir.dt.int32
AX = mybir.AxisListType
OP = mybir.AluOpType

_cached_nc = None


def _copy(use_vector, nc, out, in_):
    if use_vector:
        nc.vector.tensor_copy(out, in_)
    else:
        nc.scalar.copy(out, in_)


def _build():
    nc = bacc.Bacc("TRN2", target_bir_lowering=False, num_devices=NCORES)
    x = nc.dram_tensor("x", [BC, D], F32, kind="ExternalInput")
    wt = nc.dram_tensor("wt", [D, NO], F32, kind="ExternalInput")
    bias = nc.dram_tensor("bias", [NO], F32, kind="ExternalInput")
    idx_o = nc.dram_tensor("idx_o", [BC, 2], I32, kind="ExternalOutput")
    w_o = nc.dram_tensor("w_o", [BC, 2], F32, kind="ExternalOutput")

    x_r = x.ap().rearrange("(p s) d -> p s d", p=P)
    idx_r = idx_o.ap().rearrange("(p s) k -> p s k", p=P)
    w_r = w_o.ap().rearrange("(p s) k -> p s k", p=P)

    with tile.TileContext(nc) as tc:
        with (
            tc.tile_pool(name="singles", bufs=1) as singles,
            tc.tile_pool(name="xs_pool", bufs=3) as xs_pool,
            tc.tile_pool(name="xt_pool", bufs=3) as xt_pool,
            tc.tile_pool(name="lg_pool", bufs=3) as lg_pool,
            tc.tile_pool(name="big", bufs=1) as big,
            tc.tile_pool(name="post", bufs=1) as post,
            tc.tile_pool(name="pxt_pool", bufs=2, space="PSUM") as pxt_pool,
            tc.tile_pool(name="plg_pool", bufs=2, space="PSUM") as plg_pool,
            tc.tile_pool(name="pt2_pool", bufs=2, space="PSUM") as pt2_pool,
        ):
            ident = singles.tile([P, P], F32)
            nc.vector.memset(ident, 1.0)
            nc.gpsimd.affine_select(
                ident, ident, pattern=[[-1, P]], base=0, channel_multiplier=1,
                compare_op=OP.is_equal, fill=0.0)
            wt_sb = singles.tile([P, CH, NO], F32)
            nc.sync.dma_start(out=wt_sb, in_=wt.ap().rearrange("(c p) j -> p c j", p=P))
            bias_sb = singles.tile([1, NO], F32)
            nc.sync.dma_start(out=bias_sb, in_=bias.ap().unsqueeze(0))
            ones = singles.tile([1, 2 * P], F32)
            nc.vector.memset(ones, 1.0)
            kconst = singles.tile([P, GS], F32)
            k4const = singles.tile([P, G], F32)
            for j in range(GS):
                nc.vector.memset(kconst[:, j:j + 1], float(j))
                nc.vector.memset(k4const[:, j:j + 1], float(j * GS))

            L = big.tile([P, NHALF, 2, NO], F32)

            for blk in range(NBLK):
                xs = xs_pool.tile([P, 4, D], F32)
                nc.sync.dma_start(out=xs, in_=x_r[:, 4 * blk:4 * blk + 4, :])
                for hh in range(2):
                    h = 2 * blk + hh
                    pxt = pxt_pool.tile([P, 2 * D], F32)
                    for i in range(2):
                        for c in range(CH):
                            off = (c // 2) * 512 + (c % 2) * 256 + i * P
                            nc.tensor.transpose(
                                pxt[:, off:off + P],
                                xs[:, 2 * hh + i, c * P:(c + 1) * P], ident)
                    xt = xt_pool.tile([P, CH, 2 * P], F32)
                    for k in range(2):
                        _copy((h + k) % 2 == 0, nc,
                              xt[:, 2 * k:2 * k + 2, :].rearrange("p c n -> p (c n)"),
                              pxt[:, 512 * k:512 * (k + 1)])
                    plg = plg_pool.tile([P, 2 * P], F32)
                    for c in range(CH):
                        nc.tensor.matmul(plg[0:NO, :], wt_sb[:, c, :], xt[:, c, :],
                                         start=(c == 0), stop=False)
                    nc.tensor.matmul(plg[0:NO, :], bias_sb, ones,
                                     start=False, stop=True)
                    lgs = lg_pool.tile([NO, 2 * P], F32)
                    _copy(h % 2 == 0, nc, lgs, plg[0:NO, :])
                    pt2 = pt2_pool.tile([P, 2, NO], F32)
                    for i in range(2):
                        nc.tensor.transpose(pt2[:, i, :], lgs[:, i * P:(i + 1) * P],
                                            ident[0:NO, 0:NO])
                    _copy(h % 2 == 1, nc, L[:, h, :, :], pt2)

            LL = L[:, :, :, :].rearrange("p h i j -> p (h i) j")
            Gv = LL[:, :, 0:G]
            INv = LL[:, :, G:NO].rearrange("p s (g k) -> p s g k", g=G)

            def bcast(t):
                return t[:, :].unsqueeze(2).broadcast_to([P, S, 4])

            gmax = post.tile([P, S], F32)
            nc.vector.tensor_reduce(gmax, Gv, axis=AX.X, op=OP.max)
            eqg = post.tile([P, S, G], F32)
            nc.vector.tensor_tensor(eqg, Gv, bcast(gmax), op=OP.is_equal)
            tmp = post.tile([P, S, GS, G], F32)
            nc.vector.tensor_tensor(
                tmp.rearrange("p s k g -> p s g k"),
                eqg.unsqueeze(3).broadcast_to([P, S, G, GS]), INv, op=OP.mult)
            sel = post.tile([P, S, GS], F32)
            nc.vector.tensor_reduce(sel, tmp, axis=AX.X, op=OP.add)
            e = post.tile([P, S, GS], F32)
            nc.scalar.activation(e, sel, func=mybir.ActivationFunctionType.Exp)
            ssum = post.tile([P, S], F32)
            nc.vector.tensor_reduce(ssum, e, axis=AX.X, op=OP.add)
            rcp = post.tile([P, S], F32)
            nc.vector.reciprocal(rcp, ssum)
            pr = post.tile([P, S, GS], F32)
            nc.vector.tensor_tensor(pr, e, bcast(rcp), op=OP.mult)
            wout = post.tile([P, S, 2], F32)
            p1 = wout[:, :, 0]
            nc.vector.tensor_reduce(p1, pr, axis=AX.X, op=OP.max)
            eq1 = post.tile([P, S, GS], F32)
            nc.vector.tensor_tensor(eq1, pr, bcast(p1), op=OP.is_equal)
            tk = post.tile([P, S, GS], F32)
            kb = kconst.unsqueeze(1).broadcast_to([P, S, GS])
            nc.vector.tensor_tensor(tk, eq1, kb, op=OP.mult)
            i1 = post.tile([P, S], F32)
            nc.vector.tensor_reduce(i1, tk, axis=AX.X, op=OP.add)
            pm = post.tile([P, S, GS], F32)
            nc.vector.scalar_tensor_tensor(pm, eq1, -1e30, pr,
                                           op0=OP.mult, op1=OP.add)
            p2 = wout[:, :, 1]
            nc.vector.tensor_reduce(p2, pm, axis=AX.X, op=OP.max)
            eq2 = post.tile([P, S, GS], F32)
            nc.vector.tensor_tensor(eq2, pm, bcast(p2), op=OP.is_equal)
            tk2 = post.tile([P, S, GS], F32)
            nc.vector.tensor_tensor(tk2, eq2, kb, op=OP.mult)
            i2 = post.tile([P, S], F32)
            nc.vector.tensor_reduce(i2, tk2, axis=AX.X, op=OP.add)
            tg = post.tile([P, S, G], F32)
            nc.vector.tensor_tensor(tg, eqg,
                                    k4const.unsqueeze(1).broadcast_to([P, S, G]),
                                    op=OP.mult)
            g4 = post.tile([P, S], F32)
            nc.vector.tensor_reduce(g4, tg, axis=AX.X, op=OP.add)
            iout = post.tile([P, S, 2], I32)
            nc.vector.tensor_tensor(iout[:, :, 0], g4, i1, op=OP.add)
            nc.vector.tensor_tensor(iout[:, :, 1], g4, i2, op=OP.add)
            nc.sync.dma_start(out=idx_r, in_=iout)
            nc.sync.dma_start(out=w_r, in_=wout)
    nc.finalize()
    return nc


def _get_nc():
    global _cached_nc
    if _cached_nc is None:
        _cached_nc = _build()
    return _cached_nc


def kernel(routing_features, group_w, group_b, in_w, in_b, experts_table,
           trace=False):
    x = np.ascontiguousarray(np.asarray(routing_features, np.float32))
    gw = np.asarray(group_w, np.float32)
    gb = np.asarray(group_b, np.float32)
    iw = np.asarray(in_w, np.float32).reshape(G * GS, D)
    ib = np.asarray(in_b, np.float32).reshape(G * GS)
    table = np.asarray(experts_table, np.int32).reshape(-1)

    wt = np.ascontiguousarray(np.concatenate([gw, iw], 0).T)
    bias = np.concatenate([gb, ib], 0)

    shards = x.reshape(NCORES, BC, D)
    in_maps = [{"x": shards[c], "wt": wt, "bias": bias} for c in range(NCORES)]
    try:
        res = run_bass_kernel_spmd(_get_nc(), in_maps,
                                   core_ids=list(range(NCORES)), trace=trace)
    except (ImportError, ModuleNotFoundError):
        res = run_bass_kernel_spmd(_get_nc(), in_maps,
                                   core_ids=list(range(NCORES)), trace=False)
    idx = np.concatenate([res.results[c]["idx_o"] for c in range(NCORES)], 0)
    w = np.concatenate([res.results[c]["w_o"] for c in range(NCORES)], 0)
    expert_indices = table[idx]
    if trace:
        kernel.last_exec_time_ns = res.exec_time_ns
        if kernel.last_exec_time_ns is None:
            try:
                from concourse.timeline_sim import TimelineSim
                kernel.last_exec_time_ns = int(TimelineSim(_get_nc()).simulate())
                kernel.time_source = "cost-model timeline sim"
            except Exception:
                pass
        else:
            kernel.time_source = "ntff"
    return expert_indices, w



# revision 6
# speedup vs baseline: 3.2741x; 3.2741x over previous
import numpy as np

import concourse.bacc as bacc
import concourse.tile as tile
import concourse.mybir as mybir
from concourse.bass_utils import run_bass_kernel_spmd

B, D, G, GS = 262144, 512, 4, 4
NO = G + G * GS
NCORES = 8
BC = B // NCORES
P = 128
S = BC // P
CH = D // P
NT = 8
NBLK = S // NT

F16 = mybir.dt.float16
F32 = mybir.dt.float32
I32 = mybir.dt.int32
AX = mybir.AxisListType
OP = mybir.AluOpType

_cached_nc = None


def _build():
    nc = bacc.Bacc("TRN2", target_bir_lowering=False, num_devices=NCORES)
    x = nc.dram_tensor("x", [BC, D], F16, kind="ExternalInput")
    whl = nc.dram_tensor("whl", [2, D, NO], F16, kind="ExternalInput")
    bias = nc.dram_tensor("bias", [NO], F16, kind="ExternalInput")
    idx_o = nc.dram_tensor("idx_o", [BC, 2], I32, kind="ExternalOutput")
    w_o = nc.dram_tensor("w_o", [BC, 2], F32, kind="ExternalOutput")

    x_r = x.ap().rearrange("(p s) d -> p s d", p=P)
    idx_r = idx_o.ap().rearrange("(p s) k -> p s k", p=P)
    w_r = w_o.ap().rearrange("(p s) k -> p s k", p=P)

    with tile.TileContext(nc) as tc:
        with (
            tc.tile_pool(name="singles", bufs=1) as singles,
            tc.tile_pool(name="xs_pool", bufs=3) as xs_pool,
            tc.tile_pool(name="xt_pool", bufs=6) as xt_pool,
            tc.tile_pool(name="big", bufs=1) as big,
            tc.tile_pool(name="post", bufs=1) as post,
            tc.tile_pool(name="pxt_pool", bufs=4, space="PSUM") as pxt_pool,
            tc.tile_pool(name="pl_pool", bufs=4, space="PSUM") as pl_pool,
        ):
            ident = singles.tile([P, P], F16)
            nc.vector.memset(ident, 1.0)
            nc.gpsimd.affine_select(
                ident, ident, pattern=[[-1, P]], base=0, channel_multiplier=1,
                compare_op=OP.is_equal, fill=0.0)
            wt_sb = singles.tile([P, 2, CH, NO], F16)
            nc.sync.dma_start(
                out=wt_sb, in_=whl.ap().rearrange("h (c p) j -> p h c j", p=P))
            bias_sb = singles.tile([1, NO], F16)
            nc.sync.dma_start(out=bias_sb, in_=bias.ap().unsqueeze(0))
            ones_sb = singles.tile([1, P], F16)
            nc.vector.memset(ones_sb, 1.0)
            kconst = singles.tile([P, GS], F32)
            k4const = singles.tile([P, G], F32)
            for j in range(GS):
                nc.vector.memset(kconst[:, j:j + 1], float(j))
                nc.vector.memset(k4const[:, j:j + 1], float(j * GS))

            L = big.tile([P, S, NO], F32)

            NG = S // 2
            GPB = NT // 2
            for blk in range(NBLK):
                xs = xs_pool.tile([P, NT, D], F16)
                nc.sync.dma_start(out=xs, in_=x_r[:, NT * blk:NT * (blk + 1), :])
                for gp in range(GPB):
                    g = GPB * blk + gp
                    pxt = pxt_pool.tile([P, 2, CH, P], F16)
                    for ti in range(2):
                        for c in range(CH):
                            nc.tensor.transpose(
                                pxt[:, ti, c, :],
                                xs[:, 2 * gp + ti, c * P:(c + 1) * P], ident)
                    xt = xt_pool.tile([P, 2, CH, P], F16)
                    if g % 7 < 3:
                        nc.vector.tensor_copy(xt, pxt)
                    else:
                        nc.scalar.copy(xt, pxt)
                    pl = pl_pool.tile([P, 2, NO], F32)
                    for ti in range(2):
                        nc.tensor.matmul(pl[:, ti, :], ones_sb, bias_sb,
                                         start=True, stop=False)
                        for c in range(CH):
                            for h in range(2):
                                nc.tensor.matmul(
                                    pl[:, ti, 0:G], xt[:, ti, c, :],
                                    wt_sb[:, h, c, 0:G],
                                    start=False, stop=False)
                                nc.tensor.matmul(
                                    pl[:, ti, G:NO], xt[:, ti, c, :],
                                    wt_sb[:, h, c, G:NO],
                                    start=False, stop=(c == CH - 1 and h == 1))
                    nc.vector.tensor_copy(L[:, 2 * g:2 * g + 2, :], pl)

            iout = post.tile([P, S, 2], I32)
            wout_t = post.tile([P, S, 2], F32)

            def postprocess(s0, s1):
                SH = s1 - s0
                Gv = L[:, s0:s1, 0:G]
                INv = L[:, s0:s1, G:NO].rearrange("p s (g k) -> p s g k", g=G)

                def bcast(t):
                    return t[:, :].unsqueeze(2).broadcast_to([P, SH, 4])

                gmax = post.tile([P, SH], F32)
                nc.vector.tensor_reduce(gmax, Gv, axis=AX.X, op=OP.max)
                eqg = post.tile([P, SH, G], F32)
                nc.vector.tensor_tensor(eqg, Gv, bcast(gmax), op=OP.is_equal)
                tmp = post.tile([P, SH, GS, G], F32)
                nc.vector.tensor_tensor(
                    tmp.rearrange("p s k g -> p s g k"),
                    eqg.unsqueeze(3).broadcast_to([P, SH, G, GS]), INv,
                    op=OP.mult)
                sel = post.tile([P, SH, GS], F32)
                nc.vector.tensor_reduce(sel, tmp, axis=AX.X, op=OP.add)
                e = post.tile([P, SH, GS], F32)
                nc.scalar.activation(e, sel,
                                     func=mybir.ActivationFunctionType.Exp)
                ssum = post.tile([P, SH], F32)
                nc.vector.tensor_reduce(ssum, e, axis=AX.X, op=OP.add)
                rcp = post.tile([P, SH], F32)
                nc.vector.reciprocal(rcp, ssum)
                pr = post.tile([P, SH, GS], F32)
                nc.vector.tensor_tensor(pr, e, bcast(rcp), op=OP.mult)
                wout = wout_t[:, s0:s1, :]
                p1 = wout[:, :, 0]
                nc.vector.tensor_reduce(p1, pr, axis=AX.X, op=OP.max)
                eq1 = post.tile([P, SH, GS], F32)
                nc.vector.tensor_tensor(eq1, pr, bcast(p1), op=OP.is_equal)
                tk = post.tile([P, SH, GS], F32)
                kb = kconst.unsqueeze(1).broadcast_to([P, SH, GS])
                nc.gpsimd.tensor_tensor(tk, eq1, kb, op=OP.mult)
                i1 = post.tile([P, SH], F32)
                nc.vector.tensor_reduce(i1, tk, axis=AX.X, op=OP.add)
                pm = post.tile([P, SH, GS], F32)
                nc.vector.scalar_tensor_tensor(pm, eq1, -1e30, pr,
                                               op0=OP.mult, op1=OP.add)
                p2 = wout[:, :, 1]
                nc.vector.tensor_reduce(p2, pm, axis=AX.X, op=OP.max)
                eq2 = post.tile([P, SH, GS], F32)
                nc.vector.tensor_tensor(eq2, pm, bcast(p2), op=OP.is_equal)
                tk2 = post.tile([P, SH, GS], F32)
                nc.gpsimd.tensor_tensor(tk2, eq2, kb, op=OP.mult)
                i2 = post.tile([P, SH], F32)
                nc.vector.tensor_reduce(i2, tk2, axis=AX.X, op=OP.add)
                tg = post.tile([P, SH, G], F32)
                nc.gpsimd.tensor_tensor(
                    tg, eqg,
                    k4const.unsqueeze(1).broadcast_to([P, SH, G]), op=OP.mult)
                g4 = post.tile([P, SH], F32)
                nc.vector.tensor_reduce(g4, tg, axis=AX.X, op=OP.add)
                nc.vector.tensor_tensor(iout[:, s0:s1, 0], g4, i1, op=OP.add)
                nc.vector.tensor_tensor(iout[:, s0:s1, 1], g4, i2, op=OP.add)

            postprocess(0, S // 2)
            postprocess(S // 2, S)
            nc.sync.dma_start(out=idx_r, in_=iout)
            nc.sync.dma_start(out=w_r, in_=wout_t)
    nc.finalize()
    return nc


def _get_nc():
    global _cached_nc
    if _cached_nc is None:
        _cached_nc = _build()
    return _cached_nc


def kernel(routing_features, group_w, group_b, in_w, in_b, experts_table,
           trace=False):
    x = np.asarray(routing_features, np.float32).astype(np.float16)
    gw = np.asarray(group_w, np.float32)
    gb = np.asarray(group_b, np.float32)
    iw = np.asarray(in_w, np.float32).reshape(G * GS, D)
    ib = np.asarray(in_b, np.float32).reshape(G * GS)
    table = np.asarray(experts_table, np.int32).reshape(-1)

    wt = np.concatenate([gw, iw], 0).T.astype(np.float32)
    w_hi = wt.astype(np.float16)
    w_lo = (wt - w_hi.astype(np.float32)).astype(np.float16)
    whl = np.ascontiguousarray(np.stack([w_hi, w_lo], 0))
    bias = np.concatenate([gb, ib], 0).astype(np.float16)

    shards = np.ascontiguousarray(x.reshape(NCORES, BC, D))
    in_maps = [{"x": shards[c], "whl": whl, "bias": bias}
               for c in range(NCORES)]
    try:
        res = run_bass_kernel_spmd(_get_nc(), in_maps,
                                   core_ids=list(range(NCORES)), trace=trace)
    except (ImportError, ModuleNotFoundError):
        res = run_bass_kernel_spmd(_get_nc(), in_maps,
                                   core_ids=list(range(NCORES)), trace=False)
    idx = np.concatenate([res.results[c]["idx_o"] for c in range(NCORES)], 0)
    w = np.concatenate([res.results[c]["w_o"] for c in range(NCORES)], 0)
    expert_indices = table[idx]
    if trace:
        kernel.last_exec_time_ns = res.exec_time_ns
        if kernel.last_exec_time_ns is None:
            try:
                from concourse.timeline_sim import TimelineSim
                kernel.last_exec_time_ns = int(TimelineSim(_get_nc()).simulate())
                kernel.time_source = "cost-model timeline sim"
            except Exception:
                pass
        else:
            kernel.time_source = "ntff"
    return expert_indices, w


# revision 10
# speedup vs baseline: 3.2891x; 1.0046x over previous
import numpy as np

import concourse.bacc as bacc
import concourse.tile as tile
import concourse.mybir as mybir
from concourse.bass_utils import run_bass_kernel_spmd

B, D, G, GS = 262144, 512, 4, 4
NO = G + G * GS
NCORES = 8
BC = B // NCORES
P = 128
S = BC // P
CH = D // P
NT = 8
NBLK = S // NT
GPB = NT // 2

F16 = mybir.dt.float16
F32 = mybir.dt.float32
I32 = mybir.dt.int32
AX = mybir.AxisListType
OP = mybir.AluOpType

_cached_nc = None


def _build():
    nc = bacc.Bacc("TRN2", target_bir_lowering=False, num_devices=NCORES)
    x = nc.dram_tensor("x", [BC, D], F16, kind="ExternalInput")
    whl = nc.dram_tensor("whl", [2, D, NO], F16, kind="ExternalInput")
    bias = nc.dram_tensor("bias", [NO], F16, kind="ExternalInput")
    idx_o = nc.dram_tensor("idx_o", [BC, 2], I32, kind="ExternalOutput")
    w_o = nc.dram_tensor("w_o", [BC, 2], F32, kind="ExternalOutput")

    x_r = x.ap().rearrange("(p s) d -> p s d", p=P)
    idx_r = idx_o.ap().rearrange("(p s) k -> p s k", p=P)
    w_r = w_o.ap().rearrange("(p s) k -> p s k", p=P)

    with tile.TileContext(nc) as tc:
        with (
            tc.tile_pool(name="singles", bufs=1) as singles,
            tc.tile_pool(name="xs_pool", bufs=3) as xs_pool,
            tc.tile_pool(name="xt_pool", bufs=6) as xt_pool,
            tc.tile_pool(name="big", bufs=1) as big,
            tc.tile_pool(name="post", bufs=2) as post,
            tc.tile_pool(name="pxt_pool", bufs=4, space="PSUM") as pxt_pool,
            tc.tile_pool(name="pl_pool", bufs=4, space="PSUM") as pl_pool,
        ):
            ident = singles.tile([P, P], F16)
            nc.vector.memset(ident, 1.0)
            nc.gpsimd.affine_select(
                ident, ident, pattern=[[-1, P]], base=0, channel_multiplier=1,
                compare_op=OP.is_equal, fill=0.0)
            wt_sb = singles.tile([P, 2, CH, NO], F16)
            nc.sync.dma_start(
                out=wt_sb, in_=whl.ap().rearrange("h (c p) j -> p h c j", p=P))
            bias_sb = singles.tile([1, NO], F16)
            nc.sync.dma_start(out=bias_sb, in_=bias.ap().unsqueeze(0))
            ones_sb = singles.tile([1, P], F16)
            nc.vector.memset(ones_sb, 1.0)
            kconst = singles.tile([P, GS], F32)
            k4const = singles.tile([P, G], F32)
            for j in range(GS):
                nc.vector.memset(kconst[:, j:j + 1], float(j))
                nc.vector.memset(k4const[:, j:j + 1], float(j * GS))

            L = big.tile([P, S, NO], F32)
            iout = big.tile([P, S, 2], I32)
            wout_t = big.tile([P, S, 2], F32)

            def postprocess(s0, s1):
                SH = s1 - s0
                Gv = L[:, s0:s1, 0:G]
                INv = L[:, s0:s1, G:NO].rearrange("p s (g k) -> p s g k", g=G)

                def bcast(t):
                    return t[:, :].unsqueeze(2).broadcast_to([P, SH, 4])

                gmax = post.tile([P, SH], F32)
                nc.vector.tensor_reduce(gmax, Gv, axis=AX.X, op=OP.max)
                eqg = post.tile([P, SH, G], F32)
                nc.vector.tensor_tensor(eqg, Gv, bcast(gmax), op=OP.is_equal)
                tmp = post.tile([P, SH, GS, G], F32)
                nc.gpsimd.tensor_tensor(
                    tmp.rearrange("p s k g -> p s g k"),
                    eqg.unsqueeze(3).broadcast_to([P, SH, G, GS]), INv,
                    op=OP.mult)
                sel = post.tile([P, SH, GS], F32)
                nc.vector.tensor_reduce(sel, tmp, axis=AX.X, op=OP.add)
                s12 = post.tile([P, SH, 2], F32)
                sv1 = s12[:, :, 0]
                nc.vector.tensor_reduce(sv1, sel, axis=AX.X, op=OP.max)
                eq1 = post.tile([P, SH, GS], F32)
                nc.vector.tensor_tensor(eq1, sel, bcast(sv1), op=OP.is_equal)
                pm = post.tile([P, SH, GS], F32)
                nc.vector.scalar_tensor_tensor(pm, eq1, -1e30, sel,
                                               op0=OP.mult, op1=OP.add)
                sv2 = s12[:, :, 1]
                nc.vector.tensor_reduce(sv2, pm, axis=AX.X, op=OP.max)
                eq2 = post.tile([P, SH, GS], F32)
                nc.vector.tensor_tensor(eq2, pm, bcast(sv2), op=OP.is_equal)
                e = post.tile([P, SH, GS], F32)
                nc.scalar.activation(e, sel,
                                     func=mybir.ActivationFunctionType.Exp)
                ssum = post.tile([P, SH], F32)
                nc.vector.tensor_reduce(ssum, e, axis=AX.X, op=OP.add)
                rcp = post.tile([P, SH], F32)
                nc.vector.reciprocal(rcp, ssum)
                e12 = post.tile([P, SH, 2], F32)
                nc.scalar.activation(e12, s12,
                                     func=mybir.ActivationFunctionType.Exp)
                rcp2 = rcp[:, :].unsqueeze(2).broadcast_to([P, SH, 2])
                nc.vector.tensor_tensor(wout_t[:, s0:s1, :], e12, rcp2,
                                        op=OP.mult)
                kb = kconst.unsqueeze(1).broadcast_to([P, SH, GS])
                tk = post.tile([P, SH, GS], F32)
                nc.gpsimd.tensor_tensor(tk, eq1, kb, op=OP.mult)
                i1 = post.tile([P, SH], F32)
                nc.vector.tensor_reduce(i1, tk, axis=AX.X, op=OP.add)
                tk2 = post.tile([P, SH, GS], F32)
                nc.gpsimd.tensor_tensor(tk2, eq2, kb, op=OP.mult)
                i2 = post.tile([P, SH], F32)
                nc.vector.tensor_reduce(i2, tk2, axis=AX.X, op=OP.add)
                tg = post.tile([P, SH, G], F32)
                nc.gpsimd.tensor_tensor(
                    tg, eqg,
                    k4const.unsqueeze(1).broadcast_to([P, SH, G]), op=OP.mult)
                g4 = post.tile([P, SH], F32)
                nc.vector.tensor_reduce(g4, tg, axis=AX.X, op=OP.add)
                nc.vector.tensor_tensor(iout[:, s0:s1, 0], g4, i1, op=OP.add)
                nc.vector.tensor_tensor(iout[:, s0:s1, 1], g4, i2, op=OP.add)

            for blk in range(NBLK):
                xs = xs_pool.tile([P, NT, D], F16)
                nc.sync.dma_start(out=xs, in_=x_r[:, NT * blk:NT * (blk + 1), :])
                for gp in range(GPB):
                    g = GPB * blk + gp
                    pxt = pxt_pool.tile([P, 2, CH, P], F16)
                    for ti in range(2):
                        for c in range(CH):
                            nc.tensor.transpose(
                                pxt[:, ti, c, :],
                                xs[:, 2 * gp + ti, c * P:(c + 1) * P], ident)
                    xt = xt_pool.tile([P, 2, CH, P], F16)
                    if g % 5 < 2:
                        nc.vector.tensor_copy(xt, pxt)
                    else:
                        nc.scalar.copy(xt, pxt)
                    pl = pl_pool.tile([P, 2, NO], F32)
                    for ti in range(2):
                        nc.tensor.matmul(pl[:, ti, :], ones_sb, bias_sb,
                                         start=True, stop=False)
                        for c in range(CH):
                            for h in range(2):
                                nc.tensor.matmul(
                                    pl[:, ti, 0:G], xt[:, ti, c, :],
                                    wt_sb[:, h, c, 0:G],
                                    start=False, stop=False)
                                nc.tensor.matmul(
                                    pl[:, ti, G:NO], xt[:, ti, c, :],
                                    wt_sb[:, h, c, G:NO],
                                    start=False, stop=(c == CH - 1 and h == 1))
                    nc.vector.tensor_copy(L[:, 2 * g:2 * g + 2, :], pl)
                if blk % 8 == 7:
                    k = blk // 8
                    postprocess(64 * k, 64 * (k + 1))

            nc.sync.dma_start(out=idx_r, in_=iout)
            nc.sync.dma_start(out=w_r, in_=wout_t)
    nc.finalize()
    return nc


def _get_nc():
    global _cached_nc
    if _cached_nc is None:
        _cached_nc = _build()
    return _cached_nc


def kernel(routing_features, group_w, group_b, in_w, in_b, experts_table,
           trace=False):
    x = np.asarray(routing_features, np.float32).astype(np.float16)
    gw = np.asarray(group_w, np.float32)
    gb = np.asarray(group_b, np.float32)
    iw = np.asarray(in_w, np.float32).reshape(G * GS, D)
    ib = np.asarray(in_b, np.float32).reshape(G * GS)
    table = np.asarray(experts_table, np.int32).reshape(-1)

    wt = np.concatenate([gw, iw], 0).T.astype(np.float32)
    w_hi = wt.astype(np.float16)
    w_lo = (wt - w_hi.astype(np.float32)).astype(np.float16)
    whl = np.ascontiguousarray(np.stack([w_hi, w_lo], 0))
    bias = np.concatenate([gb, ib], 0).astype(np.float16)

    shards = np.ascontiguousarray(x.reshape(NCORES, BC, D))
    in_maps = [{"x": shards[c], "whl": whl, "bias": bias}
               for c in range(NCORES)]
    try:
        res = run_bass_kernel_spmd(_get_nc(), in_maps,
                                   core_ids=list(range(NCORES)), trace=trace)
    except (ImportError, ModuleNotFoundError):
        res = run_bass_kernel_spmd(_get_nc(), in_maps,
                                   core_ids=list(range(NCORES)), trace=False)
    idx = np.concatenate([res.results[c]["idx_o"] for c in range(NCORES)], 0)
    w = np.concatenate([res.results[c]["w_o"] for c in range(NCORES)], 0)
    expert_indices = table[idx]
    if trace:
        kernel.last_exec_time_ns = res.exec_time_ns
        if kernel.last_exec_time_ns is None:
            try:
                from concourse.timeline_sim import TimelineSim
                kernel.last_exec_time_ns = int(TimelineSim(_get_nc()).simulate())
                kernel.time_source = "cost-model timeline sim"
            except Exception:
                pass
        else:
            kernel.time_source = "ntff"
    return expert_indices, w


# revision 11
# speedup vs baseline: 3.5898x; 1.0914x over previous
import numpy as np

import concourse.bacc as bacc
import concourse.tile as tile
import concourse.mybir as mybir
from concourse.bass_utils import run_bass_kernel_spmd

B, D, G, GS = 262144, 512, 4, 4
NO = G + G * GS
NCORES = 8
BC = B // NCORES
P = 128
S = BC // P
CH = D // P
NT = 8
NBLK = S // NT
GPB = NT // 2

F16 = mybir.dt.float16
F32 = mybir.dt.float32
I32 = mybir.dt.int32
AX = mybir.AxisListType
OP = mybir.AluOpType

_cached_nc = None


def _build():
    nc = bacc.Bacc("TRN2", target_bir_lowering=False, num_devices=NCORES)
    x = nc.dram_tensor("x", [BC, D], F16, kind="ExternalInput")
    whl = nc.dram_tensor("whl", [2, D, NO], F16, kind="ExternalInput")
    bias = nc.dram_tensor("bias", [NO], F16, kind="ExternalInput")
    idx_o = nc.dram_tensor("idx_o", [BC, 2], I32, kind="ExternalOutput")
    w_o = nc.dram_tensor("w_o", [BC, 2], F32, kind="ExternalOutput")

    x_r = x.ap().rearrange("(p s) d -> p s d", p=P)
    idx_r = idx_o.ap().rearrange("(p s) k -> p s k", p=P)
    w_r = w_o.ap().rearrange("(p s) k -> p s k", p=P)

    with tile.TileContext(nc) as tc:
        with (
            tc.tile_pool(name="singles", bufs=1) as singles,
            tc.tile_pool(name="xs_pool", bufs=3) as xs_pool,
            tc.tile_pool(name="xt_pool", bufs=6) as xt_pool,
            tc.tile_pool(name="big", bufs=1) as big,
            tc.tile_pool(name="post", bufs=2) as post,
            tc.tile_pool(name="pxt_pool", bufs=4, space="PSUM") as pxt_pool,
            tc.tile_pool(name="pl_pool", bufs=4, space="PSUM") as pl_pool,
        ):
            ident = singles.tile([P, P], F16)
            nc.vector.memset(ident, 1.0)
            nc.gpsimd.affine_select(
                ident, ident, pattern=[[-1, P]], base=0, channel_multiplier=1,
                compare_op=OP.is_equal, fill=0.0)
            wt_sb = singles.tile([P, 2, CH, NO], F16)
            nc.sync.dma_start(
                out=wt_sb, in_=whl.ap().rearrange("h (c p) j -> p h c j", p=P))
            bias_sb = singles.tile([1, NO], F16)
            nc.sync.dma_start(out=bias_sb, in_=bias.ap().unsqueeze(0))
            ones_sb = singles.tile([1, P], F16)
            nc.vector.memset(ones_sb, 1.0)
            kconst = singles.tile([P, GS], F32)
            k4const = singles.tile([P, G], F32)
            for j in range(GS):
                nc.vector.memset(kconst[:, j:j + 1], float(j))
                nc.vector.memset(k4const[:, j:j + 1], float(j * GS))

            L = big.tile([P, S, NO], F32)
            iout = big.tile([P, S, 2], I32)
            wout_t = big.tile([P, S, 2], F32)

            def pp_ops(s0, s1):
                SH = s1 - s0
                Gv = L[:, s0:s1, 0:G]
                INv = L[:, s0:s1, G:NO].rearrange("p s (g k) -> p s g k", g=G)
                kb = kconst.unsqueeze(1).broadcast_to([P, SH, GS])
                k4b = k4const.unsqueeze(1).broadcast_to([P, SH, G])

                def bcast(t):
                    return t[:, :].unsqueeze(2).broadcast_to([P, SH, 4])

                v = {}

                def alloc(name, shape, dt=F32):
                    v[name] = post.tile(shape, dt, name=name)
                    return v[name]

                ops = [
                    lambda: nc.vector.tensor_reduce(
                        alloc("gmax", [P, SH]), Gv, axis=AX.X, op=OP.max),
                    lambda: nc.vector.tensor_tensor(
                        alloc("eqg", [P, SH, G]), Gv, bcast(v["gmax"]),
                        op=OP.is_equal),
                    lambda: nc.vector.tensor_tensor(
                        alloc("tmp", [P, SH, GS, G]).rearrange(
                            "p s k g -> p s g k"),
                        v["eqg"].unsqueeze(3).broadcast_to([P, SH, G, GS]),
                        INv, op=OP.mult),
                    lambda: nc.vector.tensor_reduce(
                        alloc("sel", [P, SH, GS]), v["tmp"], axis=AX.X,
                        op=OP.add),
                    lambda: nc.vector.tensor_reduce(
                        alloc("s12", [P, SH, 2])[:, :, 0], v["sel"],
                        axis=AX.X, op=OP.max),
                    lambda: nc.vector.tensor_tensor(
                        alloc("eq1", [P, SH, GS]), v["sel"],
                        bcast(v["s12"][:, :, 0]), op=OP.is_equal),
                    lambda: nc.vector.scalar_tensor_tensor(
                        alloc("pm", [P, SH, GS]), v["eq1"], -1e30, v["sel"],
                        op0=OP.mult, op1=OP.add),
                    lambda: nc.vector.tensor_reduce(
                        v["s12"][:, :, 1], v["pm"], axis=AX.X, op=OP.max),
                    lambda: nc.vector.tensor_tensor(
                        alloc("eq2", [P, SH, GS]), v["pm"],
                        bcast(v["s12"][:, :, 1]), op=OP.is_equal),
                    lambda: nc.scalar.activation(
                        alloc("e", [P, SH, GS]), v["sel"],
                        func=mybir.ActivationFunctionType.Exp),
                    lambda: nc.vector.tensor_reduce(
                        alloc("ssum", [P, SH]), v["e"], axis=AX.X, op=OP.add),
                    lambda: nc.vector.reciprocal(
                        alloc("rcp", [P, SH]), v["ssum"]),
                    lambda: nc.scalar.activation(
                        alloc("e12", [P, SH, 2]), v["s12"],
                        func=mybir.ActivationFunctionType.Exp),
                    lambda: nc.vector.tensor_tensor(
                        wout_t[:, s0:s1, :], v["e12"],
                        v["rcp"][:, :].unsqueeze(2).broadcast_to([P, SH, 2]),
                        op=OP.mult),
                    lambda: nc.gpsimd.tensor_tensor(
                        alloc("tk", [P, SH, GS]), v["eq1"], kb, op=OP.mult),
                    lambda: nc.vector.tensor_reduce(
                        alloc("i1", [P, SH]), v["tk"], axis=AX.X, op=OP.add),
                    lambda: nc.gpsimd.tensor_tensor(
                        alloc("tk2", [P, SH, GS]), v["eq2"], kb, op=OP.mult),
                    lambda: nc.vector.tensor_reduce(
                        alloc("i2", [P, SH]), v["tk2"], axis=AX.X, op=OP.add),
                    lambda: nc.gpsimd.tensor_tensor(
                        alloc("tg", [P, SH, G]), v["eqg"], k4b, op=OP.mult),
                    lambda: nc.vector.tensor_reduce(
                        alloc("g4", [P, SH]), v["tg"], axis=AX.X, op=OP.add),
                    lambda: nc.vector.tensor_tensor(
                        iout[:, s0:s1, 0], v["g4"], v["i1"], op=OP.add),
                    lambda: nc.vector.tensor_tensor(
                        iout[:, s0:s1, 1], v["g4"], v["i2"], op=OP.add),
                ]
                return ops

            pending = []
            NCH = 8
            SC = S // NCH
            PPC = SC // 2
            for blk in range(NBLK):
                xs = xs_pool.tile([P, NT, D], F16)
                nc.sync.dma_start(out=xs, in_=x_r[:, NT * blk:NT * (blk + 1), :])
                for gp in range(GPB):
                    g = GPB * blk + gp
                    pxt = pxt_pool.tile([P, 2, CH, P], F16)
                    for ti in range(2):
                        for c in range(CH):
                            nc.tensor.transpose(
                                pxt[:, ti, c, :],
                                xs[:, 2 * gp + ti, c * P:(c + 1) * P], ident)
                    xt = xt_pool.tile([P, 2, CH, P], F16)
                    if g % 5 < 2:
                        nc.vector.tensor_copy(xt, pxt)
                    else:
                        nc.scalar.copy(xt, pxt)
                    pl = pl_pool.tile([P, 2, NO], F32)
                    for ti in range(2):
                        nc.tensor.matmul(pl[:, ti, :], ones_sb, bias_sb,
                                         start=True, stop=False)
                        for c in range(CH):
                            for h in range(2):
                                nc.tensor.matmul(
                                    pl[:, ti, 0:G], xt[:, ti, c, :],
                                    wt_sb[:, h, c, 0:G],
                                    start=False, stop=False)
                                nc.tensor.matmul(
                                    pl[:, ti, G:NO], xt[:, ti, c, :],
                                    wt_sb[:, h, c, G:NO],
                                    start=False, stop=(c == CH - 1 and h == 1))
                    nc.vector.tensor_copy(L[:, 2 * g:2 * g + 2, :], pl)
                    for _ in range(2):
                        if pending:
                            pending.pop(0)()
                    if (g + 1) % PPC == 0:
                        k = (g + 1) // PPC - 1
                        pending.extend(pp_ops(SC * k, SC * (k + 1)))
                        if k == NCH - 2:
                            pass
            for op in pending:
                op()

            half = S // 2
            nc.sync.dma_start(out=idx_r[:, 0:half, :], in_=iout[:, 0:half, :])
            nc.sync.dma_start(out=w_r[:, 0:half, :], in_=wout_t[:, 0:half, :])
            nc.sync.dma_start(out=idx_r[:, half:S, :], in_=iout[:, half:S, :])
            nc.sync.dma_start(out=w_r[:, half:S, :], in_=wout_t[:, half:S, :])
    nc.finalize()
    return nc


def _get_nc():
    global _cached_nc
    if _cached_nc is None:
        _cached_nc = _build()
    return _cached_nc


def kernel(routing_features, group_w, group_b, in_w, in_b, experts_table,
           trace=False):
    x = np.asarray(routing_features, np.float32).astype(np.float16)
    gw = np.asarray(group_w, np.float32)
    gb = np.asarray(group_b, np.float32)
    iw = np.asarray(in_w, np.float32).reshape(G * GS, D)
    ib = np.asarray(in_b, np.float32).reshape(G * GS)
    table = np.asarray(experts_table, np.int32).reshape(-1)

    wt = np.concatenate([gw, iw], 0).T.astype(np.float32)
    w_hi = wt.astype(np.float16)
    w_lo = (wt - w_hi.astype(np.float32)).astype(np.float16)
    whl = np.ascontiguousarray(np.stack([w_hi, w_lo], 0))
    bias = np.concatenate([gb, ib], 0).astype(np.float16)

    shards = np.ascontiguousarray(x.reshape(NCORES, BC, D))
    in_maps = [{"x": shards[c], "whl": whl, "bias": bias}
               for c in range(NCORES)]
    try:
        res = run_bass_kernel_spmd(_get_nc(), in_maps,
                                   core_ids=list(range(NCORES)), trace=trace)
    except (ImportError, ModuleNotFoundError):
        res = run_bass_kernel_spmd(_get_nc(), in_maps,
                                   core_ids=list(range(NCORES)), trace=False)
    idx = np.concatenate([res.results[c]["idx_o"] for c in range(NCORES)], 0)
    w = np.concatenate([res.results[c]["w_o"] for c in range(NCORES)], 0)
    expert_indices = table[idx]
    if trace:
        kernel.last_exec_time_ns = res.exec_time_ns
        if kernel.last_exec_time_ns is None:
            try:
                from concourse.timeline_sim import TimelineSim
                kernel.last_exec_time_ns = int(TimelineSim(_get_nc()).simulate())
                kernel.time_source = "cost-model timeline sim"
            except Exception:
                pass
        else:
            kernel.time_source = "ntff"
    return expert_indices, w
